# revision 53
# baseline (speedup 1.0000x reference)
"""Single-head causal attention (B=8, T=4096, EMB=1024, HEAD=64) on 8 trn2 cores.

Strategy: data-parallel over batch, one batch element per NeuronCore.

Per core (all matmuls bf16 with fp32 PSUM):
  Phase 1 -- quarter-streamed QKV projection (DMA-paced):
    x^T arrives in 4 t-quarters, host-packed partition-major so each DMA
    descriptor moves 128 x 16KB contiguous rows (the DMA feed is
    row-rate-limited; 2KB rows cap at ~120GB/s vs ~350 for 16KB). Per
    quarter the PE computes KQ^T [128, 1024] (k-outer accumulation, rows
    0:64 = K^T, 64:128 = Q^T) and V [1024, 64] into PSUM; VectorE
    evacuates to SBUF bf16; a PE matmul against a 64-rolled identity
    produces kqB (partition halves swapped) for 2x row tiling -- sbuf-sbuf
    DMA would fair-share the ring with bulk x and stall stage 1.
  Phase 2 -- attention, PE in 64-row tile mode throughout (tile_position),
  staged so exp starts as soon as dependencies exist:
    Stage 0 (q0 only):  chunks a<8,  tiles j<2  -> exp from ~22us
    Stage 1 (q0-1):     chunks a<16, tiles j<4 (rest)
    Stage 2 (all):      tiles 4..7 + chunks a>=16 + PV
    Per chunk pair (a, a+1): scores S^T = K_a @ Q^T on PE tile (0,0) and
    K_{a+1} @ Q^T on tile (64,0) concurrently into multi-bank PSUM groups;
    ScalarE exps each group straight from PSUM into pt[a] bf16 (scale=1/8
    folded in); GpSimd masks the diagonal 128x128 block.
    PV computes O^T = sum_a V_a^T @ P_a^T with vt stationary (65 cols incl
    a ones column for the softmax rowsum) and pt moving (512 cols, so
    LDWEIGHTS hides), split-K across the two PE row tiles. Partial sums are
    emitted as independent PSUM sittings on a static schedule that keeps PV
    load even across pair-slots (instead of back-loaded at tile close),
    staged to SBUF and DMA'd to DRAM; the host reduces the partials,
    divides by the rowsum row and transposes.
  Engine budget: ScalarE (exp, ~8.7M elems at 1 elem/cycle/lane @1.2GHz,
  (N+352)/1.2ns per ACT) is the pacing engine; keeping PE duty high enough
  that the HAM clock gate stays at 2.4GHz is what the PV scheduling buys.
"""

from contextlib import ExitStack

import numpy as np
import ml_dtypes

B, T, EMB, HEAD = 8, 4096, 1024, 64
KCH = EMB // 128          # 8 contraction chunks
NQ = 4                    # x streamed in 4 t-quarters
QW = T // NQ              # 1024
NTT = T // 512            # 8 t-tiles of 512
NTS = T // 128            # 32 s-chunks of 128
BF16 = ml_dtypes.bfloat16

_CACHE = {}

PV_BATCH = 8
PV_SLOT_BUDGET = 16


def _pv_schedule():
    """Static per-pair-slot schedule of PV partial-sum sittings.
    Slot p may only use chunks < 2p (exp'd at least one pair earlier).
    Returns (slots, flat) where slots[p] = [(j, lo, hi), ...] and flat is
    the emission-ordered list of (j, skip) per DRAM partial (x2 halves)."""
    frontier = [0] * NTT
    slots = []
    for p in range(17):
        ready = min(2 * p, NTS) if p < 16 else NTS
        budget = PV_SLOT_BUDGET if p < 16 else 10 ** 9
        cur = []
        for j in range(NTT):
            cap = 4 * j + 4
            while frontier[j] < min(ready, cap) and budget > 0:
                lo = frontier[j]
                hi = min(lo + PV_BATCH, ready, cap)
                if hi - lo < PV_BATCH and hi < cap:
                    break
                cur.append((j, lo, hi))
                frontier[j] = hi
                budget -= hi - lo
        slots.append(cur)
    flat = []
    for cur in slots:
        for (j, lo, hi) in cur:
            skip = max(0, 128 * lo - 512 * j)
            flat.append((j, skip))
            flat.append((j, skip))
    return slots, flat


PV_SLOTS, PV_FLAT = _pv_schedule()
NSIT = len(PV_FLAT)


def _build_program():
    import concourse.bacc as bacc
    import concourse.tile as tile
    from concourse import mybir

    fp32 = mybir.dt.float32
    bf16 = mybir.dt.bfloat16
    EXP = mybir.ActivationFunctionType.Exp

    nc = bacc.Bacc("TRN2", target_bir_lowering=False, debug=False)
    # x^T quarters, partition-major so each partition's DMA row is a 16KB
    # contiguous run (the DMA feed is row-rate-limited: 2KB rows cap ~120GB/s)
    xt_ap = nc.dram_tensor(
        "xt", [NQ, 128, KCH, QW], bf16, kind="ExternalInput").ap()
    w_ap = nc.dram_tensor("w", [EMB, 192], bf16, kind="ExternalInput").ap()
    mask_ap = nc.dram_tensor("mask", [128, 128], bf16, kind="ExternalInput").ap()
    # rolled identity: rot[k, m] = 1 iff k == (m+64)%128 -> rot.T @ x rolls
    # partitions by 64 (kqB production on PE, off the congested DMA rings)
    rot_ap = nc.dram_tensor("rot", [128, 128], bf16, kind="ExternalInput").ap()
    # O^T partial sums (one [65,512] block per PV sitting, X and Y halves
    # separate); host reduces + divides by the rowsum row + transposes
    o_ap = nc.dram_tensor(
        "o", [NSIT, HEAD + 1, 512], bf16, kind="ExternalOutput").ap()

    with tile.TileContext(nc) as tc:
        with (
            tc.tile_pool(name="consts", bufs=1) as consts,
            tc.tile_pool(name="outs", bufs=4) as outs,
        ):
            # ---------- constants (allocated now, DMA'd after x q0 so the
            # x stream hits the rings first; w still lands before KQ k0) ----
            w_sb = consts.tile([128, KCH, 192], bf16, tag="w")
            mask_sb = consts.tile([128, 128], bf16, tag="mask")
            rot_sb = consts.tile([128, 128], bf16, tag="rot")
            # V with ones column: [128, 32, 65]; col 64 stays 1.0 (rowsum)
            vt_sb = consts.tile([128, NTS, 65], bf16, tag="vt")

            def emit_consts():
                for k in range(KCH):
                    nc.sync.dma_start(
                        out=w_sb[:, k, :], in_=w_ap[k * 128:(k + 1) * 128, :])
                nc.sync.dma_start(out=mask_sb, in_=mask_ap)
                nc.sync.dma_start(out=rot_sb, in_=rot_ap)
                nc.gpsimd.memset(vt_sb, 1.0)

            # kqA: rows 0:64 = K^T, 64:128 = Q^T (natural PE layout)
            # kqB: partition-halves swapped (for 2x row-tiled scores)
            kqA = consts.tile([128, T], bf16, tag="kqA")
            kqB = consts.tile([128, T], bf16, tag="kqB")

            # warm the ACT exp table set (~2.7us) off the critical path
            warm_sb = consts.tile([128, 8], fp32, tag="warm")
            nc.vector.memset(warm_sb, 0.0)
            nc.scalar.activation(warm_sb, warm_sb, EXP, scale=0.125)

            # exp'd score rows: pt[a] covers t in [128a, T)
            ptp = ExitStack()
            ptpool = ptp.enter_context(tc.tile_pool(name="pt", bufs=1))
            pt = []
            for a in range(NTS):
                pt_a = ptpool.tile([128, T - 128 * a], bf16, tag=f"pt{a}")
                pt.append(pt_a)

            # ---------- phase 1 pools ----------
            phase1 = ExitStack()
            xp = phase1.enter_context(tc.tile_pool(name="xp", bufs=2))
            ps_kq = phase1.enter_context(
                tc.tile_pool(name="ps_kq", bufs=1, space="PSUM"))
            ps_v = phase1.enter_context(
                tc.tile_pool(name="ps_v", bufs=1, space="PSUM"))
            s1stack = ExitStack()
            ps_s1 = s1stack.enter_context(
                tc.tile_pool(name="ps_s1", bufs=2, space="PSUM"))

            def emit_quarter_dma(q):
                # two descriptors per quarter (128 rows x 8KB contiguous
                # each): KQ starts on the first half; quarter q+2's first
                # half can stream as soon as q's k<4 matmuls finish
                xq = {}
                for h in range(2):
                    xh = xp.tile([128, 4, QW], bf16, tag=f"xh{h}")
                    nc.sync.dma_start(
                        out=xh, in_=xt_ap[q, :, 4 * h:4 * (h + 1), :])
                    xq[h] = xh
                return xq

            def emit_quarter_mm(q, xq, k):
                """PE work for x chunk (k, quarter q)."""
                pkq, pv = quarter_psum[q]
                for j2 in range(2):
                    nc.tensor.matmul(
                        pkq[:, j2 * 512:(j2 + 1) * 512],
                        w_sb[:, k, 0:128],
                        xq[k // 4][:, k % 4, j2 * 512:(j2 + 1) * 512],
                        start=(k == 0),
                        stop=(k == KCH - 1),
                        skip_group_check=True,
                    )
                for i in range(8):
                    nc.tensor.matmul(
                        pv[:, i, :],
                        xq[k // 4][:, k % 4, i * 128:(i + 1) * 128],
                        w_sb[:, k, 128:192],
                        start=(k == 0 and i == 0),
                        stop=(k == KCH - 1),
                        skip_group_check=True,
                    )

            def emit_quarter_evac(q):
                pkq, pv = quarter_psum[q]
                qsl = slice(q * QW, (q + 1) * QW)
                nc.vector.tensor_copy(kqA[:, qsl], pkq)
                nc.vector.tensor_copy(vt_sb[:, 8 * q:8 * q + 8, 0:64], pv)
                # partition-half swap on the PE (rolled identity), reusing the
                # kq PSUM banks; sbuf->sbuf DMA shares the ring with bulk x
                # traffic and its fair-share latency stalled stage-1 ~10us
                for j2 in range(2):
                    nc.tensor.matmul(
                        pkq[:, j2 * 512:(j2 + 1) * 512],
                        rot_sb,
                        kqA[:, q * QW + j2 * 512:q * QW + (j2 + 1) * 512],
                        start=True, stop=True,
                        skip_group_check=True,
                    )
                nc.vector.tensor_copy(kqB[:, qsl], pkq)

            # ---------- phase 2 emitters ----------
            def emit_score_group(pool, a, jstart, g, pe_tile):
                """Scores for chunk a over t-tiles [jstart, jstart+g), then
                exp straight from PSUM into pt[a]. pe_tile 0 -> PE rows 0:64
                (operands in kqA/kqB low halves), 1 -> rows 64:128."""
                ps = pool.tile([128, g * 512], fp32, tag="sg")
                if pe_tile == 0:
                    kt, qt, psl = kqA, kqB, slice(0, 64)
                else:
                    kt, qt, psl = kqB, kqA, slice(64, 128)
                for idx in range(g):
                    j = jstart + idx
                    nc.tensor.matmul(
                        ps[:, idx * 512:(idx + 1) * 512],
                        kt[psl, a * 128:(a + 1) * 128],
                        qt[psl, j * 512:(j + 1) * 512],
                        start=True, stop=True,
                        tile_position=(64 * pe_tile, 0),
                    )
                skip = max(0, 128 * a - 512 * jstart)
                nc.scalar.activation(
                    pt[a][:, 512 * jstart + skip - 128 * a:512 * (jstart + g) - 128 * a],
                    ps[:, skip:g * 512],
                    EXP,
                    scale=0.125,
                )
                if skip > 0 or 512 * jstart == 128 * a:
                    # group contains the diagonal: zero below-diagonal block
                    nc.gpsimd.tensor_mul(pt[a][:, 0:128], pt[a][:, 0:128], mask_sb)

            def emit_pair_scores(pool, p, jlo, jhi, gmax):
                a0, a1 = 2 * p, 2 * p + 1
                j = max(jlo, a0 // 4)
                while j < jhi:
                    g = min(gmax, jhi - j)
                    emit_score_group(pool, a0, j, g, pe_tile=0)
                    emit_score_group(pool, a1, j, g, pe_tile=1)
                    j += g

            # O^T partial sums: per sitting (j, chunk batch, half) the PE
            # accumulates into a PSUM bank (vt stationary, pt moving so
            # LDWEIGHTS hides) and the bank is DMA'd to DRAM; the host adds
            # the partials. This keeps PV load even across pair-slots
            # instead of back-loaded at each tile's close.
            sit_counter = [0]

            def emit_sitting(j, lo, hi):
                for (psl, tp, tag) in (
                    (slice(0, 64), (0, 0), "ox"),
                    (slice(64, 128), (64, 0), "oy"),
                ):
                    po = ps_o.tile([128, 512], fp32, tag=tag)
                    for aa in range(lo, hi):
                        skip = max(0, 128 * aa - 512 * j)
                        lo_c = 512 * j - 128 * aa + skip
                        nc.tensor.matmul(
                            po[0:65, skip:512],
                            vt_sb[psl, aa, :],
                            pt[aa][psl, lo_c:lo_c + 512 - skip],
                            start=(aa == lo), stop=(aa == hi - 1),
                            tile_position=tp,
                            skip_group_check=True,
                        )
                    st = outs.tile([65, 512], bf16, tag="st" + tag)
                    nc.vector.tensor_copy(st, po[0:65, :])
                    nc.sync.dma_start(out=o_ap[sit_counter[0]], in_=st)
                    sit_counter[0] += 1

            # ---------- emission schedule ----------
            quarter_psum = {}

            def open_quarter(q):
                pkq = ps_kq.tile([128, QW], fp32, tag="kq")
                pv = ps_v.tile([128, 8, 64], fp32, tag="v")
                quarter_psum[q] = (pkq, pv)

            # quarters arrive ~serially (xp bufs=1 staggers the DMA issue so
            # the ring round-robin doesn't make everything co-arrive late).
            # Score stages emit as soon as their kq quarters exist:
            #   stage 0 (q0 only): chunks a<8, tiles j<2 -> exp from ~17us
            #   stage 1 (q0-1):    chunks a<16, tiles j<4 (rest)
            #   stage 2 (all):     tiles 4..7 + everything a>=16 + PV
            xqs = {}
            xqs[0] = emit_quarter_dma(0)
            emit_consts()
            open_quarter(0)
            for k in range(KCH):
                emit_quarter_mm(0, xqs[0], k)
            emit_quarter_evac(0)
            xqs[1] = emit_quarter_dma(1)
            open_quarter(1)
            for k in range(KCH):
                emit_quarter_mm(1, xqs[1], k)
                if k % 2 == 1:
                    emit_pair_scores(ps_s1, k // 2, 0, 2, gmax=2)
            emit_quarter_evac(1)
            xqs[2] = emit_quarter_dma(2)
            emit_pair_scores(ps_s1, 0, 2, 4, gmax=2)
            emit_pair_scores(ps_s1, 1, 2, 4, gmax=2)
            open_quarter(2)
            for k in range(KCH):
                emit_quarter_mm(2, xqs[2], k)
                if k == 3:
                    emit_pair_scores(ps_s1, 2, 2, 4, gmax=2)
                if k == 7:
                    emit_pair_scores(ps_s1, 3, 2, 4, gmax=2)
            emit_quarter_evac(2)
            xqs[3] = emit_quarter_dma(3)
            emit_pair_scores(ps_s1, 4, 0, 4, gmax=2)
            open_quarter(3)
            for k in range(KCH):
                emit_quarter_mm(3, xqs[3], k)
                if k == 3:
                    emit_pair_scores(ps_s1, 5, 0, 4, gmax=2)
                if k == 7:
                    emit_pair_scores(ps_s1, 6, 0, 4, gmax=2)
            emit_quarter_evac(3)
            emit_pair_scores(ps_s1, 7, 0, 4, gmax=2)

            s1stack.close()
            phase1.close()

            # stage 2: remaining scores + PV, all PSUM now free
            phase2 = ExitStack()
            ps_s2 = phase2.enter_context(
                tc.tile_pool(name="ps_s2", bufs=2, space="PSUM"))
            ps_o = phase2.enter_context(
                tc.tile_pool(name="ps_o", bufs=2, space="PSUM"))
            # interleave PV sittings between score groups so the PE never
            # runs a long PV burst while ScalarE waits on scores
            for p in range(16):
                a0, a1 = 2 * p, 2 * p + 1
                sits = list(PV_SLOTS[p])
                j = max(4, a0 // 4)
                while j < NTT:
                    g = min(2, NTT - j)
                    emit_score_group(ps_s2, a0, j, g, pe_tile=0)
                    if sits:
                        emit_sitting(*sits.pop(0))
                    emit_score_group(ps_s2, a1, j, g, pe_tile=1)
                    if sits:
                        emit_sitting(*sits.pop(0))
                    j += g
                for s in sits:
                    emit_sitting(*s)
            for s in PV_SLOTS[16]:
                emit_sitting(*s)
            phase2.close()
            ptp.close()

    nc.compile()
    return nc


def _get_nc():
    if "nc" not in _CACHE:
        _CACHE["nc"] = _build_program()
    return _CACHE["nc"]


def _in_maps(x, W):
    x = np.asarray(x, dtype=np.float32)
    W = np.asarray(W, dtype=np.float32)
    assert x.shape == (B, T, EMB) and W.shape == (EMB, 3 * HEAD)

    xt = x.transpose(0, 2, 1)  # [B, EMB, T]
    # [B, NQ, 128, KCH, QW]: per (quarter, partition) a 16KB contiguous run
    xtq = np.ascontiguousarray(
        xt.reshape(B, KCH, 128, NQ, QW).transpose(0, 3, 2, 1, 4)
    ).astype(BF16)
    w16 = W.astype(BF16)
    mask = np.triu(np.ones((128, 128), np.float32)).astype(BF16)
    rot = np.zeros((128, 128), np.float32)
    rot[(np.arange(128) + 64) % 128, np.arange(128)] = 1.0
    rot = rot.astype(BF16)
    return [{"xt": xtq[b], "w": w16, "mask": mask, "rot": rot}
            for b in range(B)]


def kernel(x, W):
    from concourse.bass_utils import run_bass_kernel_spmd

    nc = _get_nc()
    res = run_bass_kernel_spmd(nc, _in_maps(x, W), list(range(B)))
    out = []
    for b in range(B):
        parts = np.asarray(res.results[b]["o"]).astype(np.float32)
        ot = np.zeros((HEAD + 1, T), np.float32)
        for s, (j, skip) in enumerate(PV_FLAT):
            ot[:, 512 * j + skip:512 * (j + 1)] += parts[s][:, skip:]
        out.append((ot[0:HEAD] / ot[HEAD:HEAD + 1]).T)  # [T, HEAD]
    return np.stack(out).astype(np.float32)
